# revision 14
# baseline (speedup 1.0000x reference)
"""Trainium2 Bass kernel for nn_CogTauBlock (CogVideoX-style transformer block).

Sharding: 8 cores = 2 batches x 4 sequence-quarters. Zero collectives: each
core redundantly computes K/V projections for its batch and processes its own
313-token query slice end-to-end (attention + FFN with full weights).

The 1250-token sequence (226 text + 1024 video) is padded to 1252 and permuted
so that every quarter has the uniform local structure [57 text | 256 video],
making the program identical across cores (SPMD).

Activations are kept feature-major ([feature, token]) on device; LayerNorm /
QK-norm / softmax statistics (partition-dim reductions) are computed with
ones/indicator matmuls on the tensor engine. RoPE is applied via host-staged
de-interleaved cos/sin tables plus 32-row partition swaps (the q/k projection
weights are row-permuted on the host to de-interleave the rotary pairs).
"""

import sys

sys.path.insert(0, "/opt/trn_rl_repo")

import numpy as np
import ml_dtypes

import concourse.bass as bass
import concourse.mybir as mybir
import concourse.tile as tile
from concourse import bacc
from concourse.bass_utils import run_bass_kernel_spmd

F32 = mybir.dt.float32
F32R = mybir.dt.float32r
BF16 = mybir.dt.bfloat16
AL = mybir.AluOpType
AF = mybir.ActivationFunctionType

# ---- problem constants ----
B = 2
DIM = 3072
HD = 64
NH = 48
TE = 226
SV = 1024
FF = 4 * DIM
S = TE + SV          # 1250 real tokens
SP = 1256            # padded (232 text slots + 1024 video); 4*314, even per-core width
TQ = SP // 4         # 314 tokens per core (f32r matmul moving dim must be even)
EPQ = 58             # text cols per quarter
KC = DIM // 128      # 24
FC = FF // 128       # 96
NCH = (SP + 127) // 128          # 10 key chunks
CHS = [128] * 9 + [SP - 9 * 128]  # [128]*9 + [104]
KCOL = [(0, 512), (512, 1024), (1024, SP)]  # k-proj/LN1 column chunks
# dummy (padding) key tokens: e-indices 226..231 -> padded positions 994..999
NDUM = 6
DUM_P0 = 3 * TQ + (226 - 3 * EPQ)  # 994
DUM_CH = DUM_P0 // 128             # chunk 7
DUM_R0 = DUM_P0 - DUM_CH * 128     # row 98
C0G = 0.7978845608028654         # sqrt(2/pi)
C1G = 0.044715
PERM64 = np.concatenate([np.arange(0, 64, 2), np.arange(1, 64, 2)])

_CACHE = {}


def _tile4(a, nj, nk):
    """[K, N] -> [nj, 128(a), nk, 128(b)] with out[j,:,k,:] = a[128k:.., 128j:..]."""
    K, N = a.shape
    assert K == nk * 128 and N == nj * 128
    return np.ascontiguousarray(
        a.reshape(nk, 128, nj, 128).transpose(2, 1, 0, 3)
    )


def _feat_cols(v, nt):
    """[nt*128] feature vector -> [128, nt] (column j = features of tile j)."""
    return np.ascontiguousarray(v.reshape(nt, 128).T)


def _host_prep(inputs):
    ins = {k: np.asarray(v, dtype=np.float32) for k, v in inputs.items()}
    hid, enc, temb = ins["hidden_states"], ins["encoder_hidden_states"], ins["temb"]
    cos, sin = ins["rope_cos"], ins["rope_sin"]

    # padded-permuted token map
    kind = np.zeros(SP, np.int64)   # 0 text, 1 video, 2 dummy
    srci = np.zeros(SP, np.int64)
    for p in range(SP):
        qq, j = divmod(p, TQ)
        if j < EPQ:
            e = EPQ * qq + j
            if e < TE:
                kind[p], srci[p] = 0, e
            else:
                kind[p], srci[p] = 2, 0
        else:
            kind[p], srci[p] = 1, 256 * qq + (j - EPQ)

    tmask = kind == 0
    vmask = kind == 1
    xT = np.zeros((B, DIM, SP), np.float32)
    for b in range(B):
        xT[b][:, tmask] = enc[b][srci[tmask]].T
        xT[b][:, vmask] = hid[b][srci[vmask]].T

    # rope tables (identity on non-video cols), de-interleaved + tiled
    rc = np.ones((SP, HD), np.float32)
    rs = np.zeros((SP, HD), np.float32)
    rc[vmask] = cos[srci[vmask]]
    rs[vmask] = sin[srci[vmask]]
    cE, cO = rc[:, 0::2].T, rc[:, 1::2].T      # [32, SP]
    sE, sO = rs[:, 0::2].T, rs[:, 1::2].T
    ropeC = np.ascontiguousarray(np.concatenate([cE, cO, cE, cO], 0))  # [128, SP]
    ropeS = np.ascontiguousarray(np.concatenate([-sE, sO, -sE, sO], 0))

    # weights (q/k row-permuted to de-interleave rotary pairs)
    permF = (np.arange(NH)[:, None] * HD + PERM64[None, :]).ravel()
    wqT = _tile4(ins["wq"][permF].T, KC, KC).astype(ml_dtypes.bfloat16)
    wkT = _tile4(ins["wk"][permF].T, KC, KC).astype(ml_dtypes.bfloat16)
    woT = _tile4(ins["wo"].T, KC, KC).astype(ml_dtypes.bfloat16)
    # wv: [6(c), 128(a), 24(k), 512(n)] column-chunk layout
    wvt = ins["wv"].T.reshape(KC, 128, 6, 512).transpose(2, 1, 0, 3)
    wvT = np.ascontiguousarray(wvt).astype(ml_dtypes.bfloat16)
    ff1T = _tile4(ins["w_ff1"].T, FC, KC)  # [96, 128, 24, 128] f32
    # ff2: [4(g), 96(k), 128(a), 6(i), 128(b)]
    f2 = ins["w_ff2"].T.reshape(FC, 128, 4, 6, 128).transpose(2, 0, 1, 3, 4)
    ff2T = np.ascontiguousarray(f2)

    # adaln modulation folded with LN affine (host: tiny temb path, replicated)
    def sigm(x):
        return 1.0 / (1.0 + np.exp(-x))

    st = temb * sigm(temb)
    cm = np.zeros((B, 14, DIM), np.float32)
    for lay, (wa, ba, g, bb) in enumerate(
        [
            (ins["w_adaln1"], ins["b_adaln1"], ins["ln1_g"], ins["ln1_b"]),
            (ins["w_adaln2"], ins["b_adaln2"], ins["ln2_g"], ins["ln2_b"]),
        ]
    ):
        mods = st @ wa.T + ba
        sh, sc, gt, esh, esc, egt = np.split(mods, 6, axis=-1)
        o = lay * 6
        cm[:, o + 0] = g * (1 + esc) ; cm[:, o + 1] = bb * (1 + esc) + esh  # e scale/shift
        cm[:, o + 2] = g * (1 + sc)  ; cm[:, o + 3] = bb * (1 + sc) + sh    # x scale/shift
        cm[:, o + 4] = gt            ; cm[:, o + 5] = egt                   # gates x/e
    cm[:, 12] = ins["bo"][None, :]
    cm[:, 13] = ins["b_ff2"][None, :]
    # -> [B, 128, 24, 14]
    cmods = np.ascontiguousarray(cm.reshape(B, 14, KC, 128).transpose(0, 3, 2, 1))
    bff1c = _feat_cols(ins["b_ff1"], FC)

    nqg = np.tile(ins["nq_g"][PERM64], 2).reshape(128, 1).astype(np.float32)
    nqb = np.tile(ins["nq_b"][PERM64], 2).reshape(128, 1).astype(np.float32)
    nkg = np.tile(ins["nk_g"][PERM64], 2).reshape(128, 1).astype(np.float32)
    nkb = np.tile(ins["nk_b"][PERM64], 2).reshape(128, 1).astype(np.float32)

    dmask = np.zeros((128, 1), np.float32)
    dmask[DUM_R0 : DUM_R0 + NDUM, 0] = -30000.0
    ind2 = np.zeros((128, 2), np.float32)
    ind2[:64, 0] = 1.0
    ind2[64:, 1] = 1.0
    ind2T = np.ascontiguousarray(ind2.T)
    ones = np.ones((128, 128), np.float32)

    in_maps = []
    for c in range(8):
        b, qq = divmod(c, 4)
        in_maps.append(
            dict(
                xT=xT[b],
                xTown=np.ascontiguousarray(xT[b][:, qq * TQ : (qq + 1) * TQ]),
                ropeCk=ropeC,
                ropeSk=ropeS,
                ropeCq=np.ascontiguousarray(ropeC[:, qq * TQ : (qq + 1) * TQ]),
                ropeSq=np.ascontiguousarray(ropeS[:, qq * TQ : (qq + 1) * TQ]),
                wqT=wqT, wkT=wkT, wvT=wvT, woT=woT, ff1T=ff1T, ff2T=ff2T,
                cmods=cmods[b], bff1c=bff1c,
                nqg=nqg, nqb=nqb, nkg=nkg, nkb=nkb,
                ind2=ind2, ind2T=ind2T, ones=ones, dmask=dmask,
            )
        )
    meta = dict(kind=kind, srci=srci)
    return in_maps, meta


def _build_program():
    nc = bacc.Bacc()
    P = lambda name, sh, dt: nc.declare_dram_parameter(name, list(sh), dt, isOutput=False)
    xT = P("xT", (DIM, SP), F32)
    xTown = P("xTown", (DIM, TQ), F32)
    ropeCk = P("ropeCk", (128, SP), F32)
    ropeSk = P("ropeSk", (128, SP), F32)
    ropeCq = P("ropeCq", (128, TQ), F32)
    ropeSq = P("ropeSq", (128, TQ), F32)
    wqT = P("wqT", (KC, 128, KC, 128), BF16)
    wkT = P("wkT", (KC, 128, KC, 128), BF16)
    wvT = P("wvT", (6, 128, KC, 512), BF16)
    woT = P("woT", (KC, 128, KC, 128), BF16)
    ff1T = P("ff1T", (FC, 128, KC, 128), F32)
    ff2T = P("ff2T", (4, FC, 128, 6, 128), F32)
    cmods = P("cmods", (128, KC, 14), F32)
    bff1c = P("bff1c", (128, FC), F32)
    nqg = P("nqg", (128, 1), F32)
    nqb = P("nqb", (128, 1), F32)
    nkg = P("nkg", (128, 1), F32)
    nkb = P("nkb", (128, 1), F32)
    dmaskd = P("dmask", (128, 1), F32)
    ind2d = P("ind2", (128, 2), F32)
    ind2Td = P("ind2T", (2, 128), F32)
    onesd = P("ones", (128, 128), F32)
    outT = nc.declare_dram_parameter("outT", [DIM, TQ], F32, isOutput=True)

    lp = nc.allow_low_precision(reason="float32r rounding is intended")
    lp.__enter__()
    with tile.TileContext(nc) as tc:
        consts = tc.alloc_tile_pool(name="consts", bufs=1)
        dram = tc.alloc_tile_pool(name="dram", bufs=1, space="DRAM")

        ones = consts.tile([128, 128], F32R)
        nc.sync.dma_start(out=ones, in_=onesd[:].bitcast(F32R))
        ind2 = consts.tile([128, 2], F32R)
        nc.sync.dma_start(out=ind2, in_=ind2d[:].bitcast(F32R))
        ind2T = consts.tile([2, 128], F32R)
        nc.sync.dma_start(out=ind2T, in_=ind2Td[:].bitcast(F32R))
        cmod = consts.tile([128, KC, 14], F32)
        nc.sync.dma_start(out=cmod, in_=cmods[:])
        bff1 = consts.tile([128, FC], F32)
        nc.sync.dma_start(out=bff1, in_=bff1c[:])
        rCk = consts.tile([128, SP], F32)
        nc.sync.dma_start(out=rCk, in_=ropeCk[:])
        rSk = consts.tile([128, SP], F32)
        nc.sync.dma_start(out=rSk, in_=ropeSk[:])
        rCq = consts.tile([128, TQ], F32)
        nc.sync.dma_start(out=rCq, in_=ropeCq[:])
        rSq = consts.tile([128, TQ], F32)
        nc.sync.dma_start(out=rSq, in_=ropeSq[:])
        vg = {}
        for nm, d in (("nqg", nqg), ("nqb", nqb), ("nkg", nkg), ("nkb", nkb)):
            t = consts.tile([128, 1], F32, name=nm, tag=nm)
            nc.sync.dma_start(out=t, in_=d[:])
            vg[nm] = t
        dmask = consts.tile([128, 1], F32)
        nc.sync.dma_start(out=dmask, in_=dmaskd[:])
        eps1 = consts.tile([1, 1], F32)
        nc.vector.memset(eps1, 1e-5)
        eps2 = consts.tile([2, 1], F32)
        nc.vector.memset(eps2, 1e-6)

        # persistent sbuf tensors (phase 0-A)
        persist1 = tc.alloc_tile_pool(name="persist1", bufs=1)
        catn1T = persist1.tile([128, KC, SP], BF16)
        catn1Tq = persist1.tile([128, KC, TQ], BF16)

        # dram spills
        qsp = dram.tile([KC, 128, TQ], BF16)
        ksp = dram.tile([KC, 128, SP], BF16)
        vsp = dram.tile([NCH, 128, NH, HD], BF16)
        gsp = dram.tile([FC, 128, TQ], F32)

        # =========== Phase 0: LN1 (full seq -> catn1T, own slice -> catn1Tq)
        with tc.tile_pool(name="p0", bufs=2) as p0, \
             tc.tile_pool(name="p0ps", bufs=1, space="PSUM") as p0ps:

            def ln_phase(src_dram, width, colchunks, dst, nq):
                # stats
                psx = [p0ps.tile([1, c1 - c0], F32, name=f"psx{i}", tag=f"sx{i}") for i, (c0, c1) in enumerate(colchunks)]
                psq = [p0ps.tile([1, c1 - c0], F32, name=f"psq{i}", tag=f"sq{i}") for i, (c0, c1) in enumerate(colchunks)]
                for k in range(KC):
                    xk = p0.tile([128, width], F32R, tag="xk")
                    nc.sync.dma_start(out=xk, in_=src_dram[k * 128 : (k + 1) * 128, :].bitcast(F32R))
                    sqk = p0.tile([128, width], F32R, tag="sqk")
                    nc.scalar.square(sqk, xk[:])
                    for i, (c0, c1) in enumerate(colchunks):
                        nc.tensor.matmul(psx[i], ones[:, 0:1], xk[:, c0:c1], start=(k == 0), stop=(k == KC - 1))
                        nc.tensor.matmul(psq[i], ones[:, 0:1], sqk[:, c0:c1], start=(k == 0), stop=(k == KC - 1))
                mrep = p0.tile([128, width], F32, tag="mrep", bufs=1)
                rrep = p0.tile([128, width], F32, tag="rrep", bufs=1)
                for i, (c0, c1) in enumerate(colchunks):
                    w = c1 - c0
                    m = p0.tile([1, w], F32R, tag="m")
                    nc.vector.tensor_scalar_mul(m, psx[i], 1.0 / DIM)
                    msq = p0.tile([1, w], F32, tag="msq")
                    nc.vector.tensor_mul(msq, m[:], m[:])
                    var = p0.tile([1, w], F32, tag="var")
                    nc.vector.scalar_tensor_tensor(
                        out=var, in0=psq[i], scalar=1.0 / DIM, in1=msq[:], op0=AL.mult, op1=AL.subtract
                    )
                    sd = p0.tile([1, w], F32, tag="sd")
                    nc.scalar.activation(sd, var[:], AF.Sqrt, bias=eps1[0:1, :])
                    rs = p0.tile([1, w], F32R, tag="rs")
                    nc.vector.reciprocal(rs, sd[:])
                    pm = p0ps.tile([128, w], F32, tag="rep")
                    nc.tensor.matmul(pm, ones[0:1, :], m[:], start=True, stop=True)
                    nc.vector.tensor_copy(mrep[:, c0:c1], pm[:])
                    pr = p0ps.tile([128, w], F32, tag="rep")
                    nc.tensor.matmul(pr, ones[0:1, :], rs[:], start=True, stop=True)
                    nc.vector.tensor_copy(rrep[:, c0:c1], pr[:])
                nquart = width // TQ
                for k in range(KC):
                    xk = p0.tile([128, width], F32, tag="xk2")
                    nc.sync.dma_start(out=xk, in_=src_dram[k * 128 : (k + 1) * 128, :])
                    t = p0.tile([128, width], F32, tag="t")
                    nc.vector.tensor_sub(t, xk[:], mrep[:])
                    t2 = p0.tile([128, width], F32, tag="t2")
                    nc.vector.tensor_mul(t2, t[:], rrep[:])
                    for qq in range(nquart):
                        for (r0, r1, ci) in (
                            (qq * TQ, qq * TQ + EPQ, 0),
                            (qq * TQ + EPQ, (qq + 1) * TQ, 2),
                        ):
                            nc.vector.scalar_tensor_tensor(
                                out=dst[:, k, r0:r1],
                                in0=t2[:, r0:r1],
                                scalar=cmod[:, k, ci : ci + 1],
                                in1=cmod[:, k, ci + 1 : ci + 2].to_broadcast((128, r1 - r0)),
                                op0=AL.mult,
                                op1=AL.add,
                            )

            ln_phase(xT, SP, KCOL, catn1T, 4)
            ln_phase(xTown, TQ, [(0, TQ)], catn1Tq, 1)

        # =========== Phase A: QKV projections (+qknorm+rope, spill)
        with tc.tile_pool(name="pa", bufs=1) as pa, \
             tc.tile_pool(name="paw", bufs=2) as paw, \
             tc.tile_pool(name="paps", bufs=2, space="PSUM") as paps:

            def qk_epilogue(ps, w, rC, rS, g, b, dst):
                raw = pa.tile([128, w], F32R, tag="e_raw", bufs=3)
                nc.vector.tensor_copy(raw, ps)
                sq = pa.tile([128, w], F32R, tag="e_A", bufs=2)
                nc.scalar.square(sq, raw[:])
                stx = paps.tile([2, w], F32, tag="st", bufs=2)
                nc.tensor.matmul(stx, ind2[:], raw[:], start=True, stop=True)
                stq = paps.tile([2, w], F32, tag="st", bufs=2)
                nc.tensor.matmul(stq, ind2[:], sq[:], start=True, stop=True)
                m = pa.tile([2, w], F32R, tag="e_B", bufs=2)
                nc.vector.tensor_scalar_mul(m, stx, 1.0 / HD)
                msq = pa.tile([2, w], F32, tag="e_C", bufs=2)
                nc.vector.tensor_mul(msq, m[:], m[:])
                var = pa.tile([2, w], F32, tag="e_D", bufs=2)
                nc.vector.scalar_tensor_tensor(
                    out=var, in0=stq, scalar=1.0 / HD, in1=msq[:], op0=AL.mult, op1=AL.subtract
                )
                sd = pa.tile([2, w], F32, tag="e_E", bufs=2)
                nc.scalar.activation(sd, var[:], AF.Sqrt, bias=eps2[:])
                rs = pa.tile([2, w], F32R, tag="e_F", bufs=2)
                nc.vector.reciprocal(rs, sd[:])
                pm = paps.tile([128, w], F32, tag="erep", bufs=2)
                nc.tensor.matmul(pm, ind2T[:], m[:], start=True, stop=True)
                pr = paps.tile([128, w], F32, tag="erep", bufs=2)
                nc.tensor.matmul(pr, ind2T[:], rs[:], start=True, stop=True)
                t = pa.tile([128, w], F32, tag="e_A", bufs=2)
                nc.vector.tensor_sub(t, raw[:], pm)
                t2 = pa.tile([128, w], F32, tag="e_B", bufs=2)
                nc.vector.tensor_mul(t2, t[:], pr)
                xn = pa.tile([128, w], F32, tag="e_C", bufs=2)
                nc.vector.scalar_tensor_tensor(
                    out=xn, in0=t2[:], scalar=g, in1=b.to_broadcast((128, w)), op0=AL.mult, op1=AL.add
                )
                sw = pa.tile([128, w], F32, tag="e_D", bufs=2)
                for b0 in (0, 64):
                    nc.vector.tensor_copy(sw[b0 : b0 + 32, :], xn[b0 + 32 : b0 + 64, :])
                    nc.vector.tensor_copy(sw[b0 + 32 : b0 + 64, :], xn[b0 : b0 + 32, :])
                p1 = pa.tile([128, w], F32, tag="e_E", bufs=2)
                nc.vector.tensor_mul(p1, xn[:], rC)
                p2 = pa.tile([128, w], F32, tag="e_F", bufs=2)
                nc.vector.tensor_mul(p2, sw[:], rS)
                ob = pa.tile([128, w], BF16, tag="e_ob", bufs=2)
                nc.vector.tensor_add(ob, p1[:], p2[:])
                nc.sync.dma_start(out=dst, in_=ob)

            for j in range(KC):
                wq_s = paw.tile([128, KC, 128], BF16, tag="wslab")
                nc.sync.dma_start(out=wq_s, in_=wqT[j])
                psq = paps.tile([128, TQ], F32, tag="mm", bufs=3)
                for k in range(KC):
                    nc.tensor.matmul(psq, wq_s[:, k, :], catn1Tq[:, k, :], start=(k == 0), stop=(k == KC - 1))
                qk_epilogue(psq[:], TQ, rCq[:], rSq[:], vg["nqg"][:], vg["nqb"][:], qsp[j])

                wk_s = paw.tile([128, KC, 128], BF16, tag="wslab")
                nc.sync.dma_start(out=wk_s, in_=wkT[j])
                for (c0, c1) in KCOL:
                    psk = paps.tile([128, c1 - c0], F32, tag="mm", bufs=3)
                    for k in range(KC):
                        nc.tensor.matmul(psk, wk_s[:, k, :], catn1T[:, k, c0:c1], start=(k == 0), stop=(k == KC - 1))
                    qk_epilogue(psk[:], c1 - c0, rCk[:, c0:c1], rSk[:, c0:c1], vg["nkg"][:], vg["nkb"][:], ksp[j][:, c0:c1])

            # V: token-major, per weight column-chunk
            for ci in range(6):
                wv_s = paw.tile([128, KC, 512], BF16, tag="vslab", bufs=1)
                nc.sync.dma_start(out=wv_s, in_=wvT[ci])
                for tt in range(NCH):
                    rows = CHS[tt]
                    psv = paps.tile([128, 512], F32, tag="mm", bufs=3)
                    for k in range(KC):
                        nc.tensor.matmul(
                            psv[0:rows, :],
                            catn1T[:, k, tt * 128 : tt * 128 + rows],
                            wv_s[:, k, :],
                            start=(k == 0),
                            stop=(k == KC - 1),
                        )
                    vb = pa.tile([128, 512], BF16, tag="vb", bufs=3)
                    nc.vector.tensor_copy(vb[0:rows, :], psv[0:rows, :])
                    nc.sync.dma_start(out=vsp[tt, 0:rows, ci * 8 : (ci + 1) * 8, :], in_=vb[0:rows, :])

        persist1.release()
        # persistent sbuf tensors (phase B-E)
        persist2 = tc.alloc_tile_pool(name="persist2", bufs=1)
        oT = persist2.tile([128, KC, TQ], BF16)
        hT = persist2.tile([128, KC, TQ], F32R)
        xn2T = persist2.tile([128, KC, TQ], F32R)

        # =========== Phase B: attention per head-pair + O-projection
        with tc.tile_pool(name="pb", bufs=1) as pb, \
             tc.tile_pool(name="pbps", bufs=1, space="PSUM") as pbps:
            for j in range(KC):
                kTp = pb.tile([128, SP], BF16, tag="kTp", bufs=2)
                nc.sync.dma_start(out=kTp, in_=ksp[j])
                qTp = pb.tile([128, TQ], BF16, tag="qTp", bufs=2)
                nc.sync.dma_start(out=qTp, in_=qsp[j])
                vp = pb.tile([128, NCH, 2, HD + 1], BF16, tag="vp", bufs=2)
                for hh in range(2):
                    nc.sync.dma_start(
                        out=vp[:, :, hh, 0:HD],
                        in_=vsp[:, :, 2 * j + hh, :].rearrange("c p d -> p c d"),
                    )
                nc.vector.memset(vp[:, :, :, HD : HD + 1], 1.0)
                Pa = pb.tile([128, NCH, TQ], BF16, tag="Pa", bufs=2)
                Pb = pb.tile([128, NCH, TQ], BF16, tag="Pb", bufs=2)
                for c in range(NCH):
                    rows = CHS[c]
                    c0 = c * 128
                    psa = pbps.tile([128, TQ], F32, tag="sc", bufs=2)
                    nc.tensor.matmul(psa[0:rows, :], kTp[0:64, c0 : c0 + rows], qTp[0:64, :], start=True, stop=True)
                    psb = pbps.tile([128, TQ], F32, tag="sc", bufs=2)
                    nc.tensor.matmul(psb[0:rows, :], kTp[64:128, c0 : c0 + rows], qTp[64:128, :], start=True, stop=True)
                    mb = dmask[0:rows, :] if c == DUM_CH else 0.0
                    nc.scalar.activation(Pa[0:rows, c, :], psa[0:rows, :], AF.Exp, scale=0.125, bias=mb)
                    nc.scalar.activation(Pb[0:rows, c, :], psb[0:rows, :], AF.Exp, scale=0.125, bias=mb)
                for h, Ph in ((0, Pa), (1, Pb)):
                    pso = pbps.tile([HD + 1, TQ], F32, tag="pso", bufs=2)
                    for c in range(NCH):
                        rows = CHS[c]
                        nc.tensor.matmul(
                            pso, vp[0:rows, c, h, :], Ph[0:rows, c, :], start=(c == 0), stop=(c == NCH - 1)
                        )
                    den = pb.tile([1, TQ], F32, tag="den")
                    nc.vector.tensor_copy(den, pso[HD : HD + 1, :])
                    rec = pb.tile([1, TQ], F32R, tag="rec")
                    nc.vector.reciprocal(rec, den[:])
                    psr = pbps.tile([HD, TQ], F32, tag="psr", bufs=1)
                    nc.tensor.matmul(psr, ones[0:1, 0:HD], rec[:], start=True, stop=True)
                    oc = pb.tile([HD, TQ], F32, tag="oc")
                    nc.vector.tensor_copy(oc, pso[0:HD, :])
                    nc.vector.tensor_mul(oT[HD * h : HD * (h + 1), j, :], oc[:], psr[:])

            # O-projection + gated residual
            for i in range(KC):
                wo_s = pb.tile([128, KC, 128], BF16, tag="woslab", bufs=2)
                nc.sync.dma_start(out=wo_s, in_=woT[i])
                pso2 = pbps.tile([128, TQ], F32, tag="mm2", bufs=2)
                for k in range(KC):
                    nc.tensor.matmul(pso2, wo_s[:, k, :], oT[:, k, :], start=(k == 0), stop=(k == KC - 1))
                xo = pb.tile([128, TQ], F32, tag="xo", bufs=2)
                nc.sync.dma_start(out=xo, in_=xTown[i * 128 : (i + 1) * 128, :])
                tb = pb.tile([128, TQ], F32, tag="tbo")
                nc.scalar.activation(tb, pso2[:], AF.Identity, bias=cmod[:, i, 12:13])
                for (r0, r1, gi) in ((0, EPQ, 5), (EPQ, TQ, 4)):
                    nc.vector.scalar_tensor_tensor(
                        out=hT[:, i, r0:r1],
                        in0=tb[:, r0:r1],
                        scalar=cmod[:, i, gi : gi + 1],
                        in1=xo[:, r0:r1],
                        op0=AL.mult,
                        op1=AL.add,
                    )

        # =========== Phase C: LN2 -> xn2T
        with tc.tile_pool(name="pc", bufs=2) as pc, \
             tc.tile_pool(name="pcps", bufs=1, space="PSUM") as pcps:
            psx = pcps.tile([1, TQ], F32, tag="sx")
            psq2 = pcps.tile([1, TQ], F32, tag="sq")
            for k in range(KC):
                sq2 = pc.tile([128, TQ], F32R, tag="csq")
                nc.scalar.square(sq2, hT[:, k, :])
                nc.tensor.matmul(psx, ones[:, 0:1], hT[:, k, :], start=(k == 0), stop=(k == KC - 1))
                nc.tensor.matmul(psq2, ones[:, 0:1], sq2[:], start=(k == 0), stop=(k == KC - 1))
            m2 = pc.tile([1, TQ], F32R, tag="m2")
            nc.vector.tensor_scalar_mul(m2, psx, 1.0 / DIM)
            msq2 = pc.tile([1, TQ], F32, tag="msq2")
            nc.vector.tensor_mul(msq2, m2[:], m2[:])
            var2 = pc.tile([1, TQ], F32, tag="var2")
            nc.vector.scalar_tensor_tensor(
                out=var2, in0=psq2, scalar=1.0 / DIM, in1=msq2[:], op0=AL.mult, op1=AL.subtract
            )
            sd2 = pc.tile([1, TQ], F32, tag="sd2")
            nc.scalar.activation(sd2, var2[:], AF.Sqrt, bias=eps1[0:1, :])
            rs2 = pc.tile([1, TQ], F32R, tag="rs2")
            nc.vector.reciprocal(rs2, sd2[:])
            pm2 = pcps.tile([128, TQ], F32, tag="rep2")
            nc.tensor.matmul(pm2, ones[0:1, :], m2[:], start=True, stop=True)
            pr2 = pcps.tile([128, TQ], F32, tag="rep2")
            nc.tensor.matmul(pr2, ones[0:1, :], rs2[:], start=True, stop=True)
            mr2 = pc.tile([128, TQ], F32, tag="mr2", bufs=1)
            nc.vector.tensor_copy(mr2, pm2[:])
            rr2 = pc.tile([128, TQ], F32, tag="rr2", bufs=1)
            nc.vector.tensor_copy(rr2, pr2[:])
            for k in range(KC):
                t = pc.tile([128, TQ], F32, tag="ct")
                nc.vector.tensor_sub(t, hT[:, k, :], mr2[:])
                t2 = pc.tile([128, TQ], F32, tag="ct2")
                nc.vector.tensor_mul(t2, t[:], rr2[:])
                for (r0, r1, ci) in ((0, EPQ, 6), (EPQ, TQ, 8)):
                    nc.vector.scalar_tensor_tensor(
                        out=xn2T[:, k, r0:r1],
                        in0=t2[:, r0:r1],
                        scalar=cmod[:, k, ci : ci + 1],
                        in1=cmod[:, k, ci + 1 : ci + 2].to_broadcast((128, r1 - r0)),
                        op0=AL.mult,
                        op1=AL.add,
                    )

        # =========== Phase D: FF1 + gelu(tanh) -> gsp
        with tc.tile_pool(name="pd", bufs=2) as pd, \
             tc.tile_pool(name="pdw", bufs=2) as pdw, \
             tc.tile_pool(name="pdps", bufs=2, space="PSUM") as pdps:
            for f in range(FC):
                f1_s = pdw.tile([128, KC, 128], F32R, tag="f1slab")
                nc.sync.dma_start(out=f1_s, in_=ff1T[f].bitcast(F32R))
                psf = pdps.tile([128, TQ], F32, tag="mm")
                for k in range(KC):
                    nc.tensor.matmul(psf, f1_s[:, k, :], xn2T[:, k, :], start=(k == 0), stop=(k == KC - 1))
                xb = pd.tile([128, TQ], F32, tag="xb")
                nc.scalar.activation(xb, psf[:], AF.Identity, bias=bff1[:, f : f + 1])
                sq = pd.tile([128, TQ], F32, tag="gsq")
                nc.scalar.square(sq, xb[:])
                t = pd.tile([128, TQ], F32, tag="gt1")
                nc.vector.tensor_scalar(out=t, in0=sq[:], scalar1=C0G * C1G, scalar2=C0G, op0=AL.mult, op1=AL.add)
                u = pd.tile([128, TQ], F32, tag="gu")
                nc.vector.tensor_mul(u, t[:], xb[:])
                th = pd.tile([128, TQ], F32, tag="gth")
                nc.scalar.activation(th, u[:], AF.Tanh)
                v = pd.tile([128, TQ], F32, tag="gv")
                nc.vector.tensor_scalar(out=v, in0=th[:], scalar1=0.5, scalar2=0.5, op0=AL.mult, op1=AL.add)
                g = pd.tile([128, TQ], F32R, tag="gg")
                nc.vector.tensor_mul(g, v[:], xb[:])
                nc.sync.dma_start(out=gsp[f].bitcast(F32R), in_=g)

        # =========== Phase E: FF2 + gated residual -> outT
        with tc.tile_pool(name="pe", bufs=1) as pe, \
             tc.tile_pool(name="peps", bufs=1, space="PSUM") as peps:
            for gi in range(4):
                psg = [peps.tile([128, TQ], F32, name=f"psg{ii}", tag=f"eg{ii}") for ii in range(6)]
                for k in range(FC):
                    gk = pe.tile([128, TQ], F32R, tag="gk", bufs=3)
                    nc.sync.dma_start(out=gk, in_=gsp[k].bitcast(F32R))
                    f2_s = pe.tile([128, 6, 128], F32R, tag="f2slab", bufs=3)
                    nc.sync.dma_start(out=f2_s, in_=ff2T[gi, k].bitcast(F32R))
                    for ii in range(6):
                        nc.tensor.matmul(psg[ii], f2_s[:, ii, :], gk[:], start=(k == 0), stop=(k == FC - 1))
                for ii in range(6):
                    i = gi * 6 + ii
                    tb = pe.tile([128, TQ], F32, tag="etb", bufs=2)
                    nc.scalar.activation(tb, psg[ii][:], AF.Identity, bias=cmod[:, i, 13:14])
                    osb = pe.tile([128, TQ], F32, tag="osb", bufs=2)
                    for (r0, r1, gix) in ((0, EPQ, 5 + 6), (EPQ, TQ, 4 + 6)):
                        nc.vector.scalar_tensor_tensor(
                            out=osb[:, r0:r1],
                            in0=tb[:, r0:r1],
                            scalar=cmod[:, i, gix : gix + 1],
                            in1=hT[:, i, r0:r1],
                            op0=AL.mult,
                            op1=AL.add,
                        )
                    nc.sync.dma_start(out=outT[i * 128 : (i + 1) * 128, :], in_=osb)

        persist2.release()
        dram.release()
        consts.release()

    lp.__exit__(None, None, None)
    nc.compile()
    return nc


def _get_program():
    if "nc" not in _CACHE:
        _CACHE["nc"] = _build_program()
    return _CACHE["nc"]


def _run(inputs, trace=False):
    in_maps, meta = _host_prep(inputs)
    nc = _get_program()
    res = run_bass_kernel_spmd(nc, in_maps, core_ids=list(range(8)), trace=trace)
    kind, srci = meta["kind"], meta["srci"]
    h = np.zeros((B, SV, DIM), np.float32)
    e = np.zeros((B, TE, DIM), np.float32)
    for b in range(B):
        full = np.concatenate(
            [res.results[b * 4 + qq]["outT"] for qq in range(4)], axis=1
        )  # [DIM, SP]
        tm, vm = kind == 0, kind == 1
        e[b][srci[tm]] = full[:, tm].T
        h[b][srci[vm]] = full[:, vm].T
    return (h, e), res


def _bench(inputs, iters=20):
    """Time repeated on-device executions (inputs stay device-resident;
    donated zero output buffers re-staged per call). Returns list of secs."""
    import time
    import jax
    import jax.numpy as jnp
    from jax.sharding import Mesh, PartitionSpec, NamedSharding
    from jax.experimental.shard_map import shard_map
    from concourse import bass2jax, mybir as _mybir

    in_maps, _ = _host_prep(inputs)
    nc = _get_program()
    bass2jax.install_neuronx_cc_hook()
    n_cores = 8
    in_names, out_names, out_avals, zero_outs = [], [], [], []
    for alloc in nc.m.functions[0].allocations:
        if not isinstance(alloc, _mybir.MemoryLocationSet):
            continue
        name = alloc.memorylocations[0].name
        if alloc.kind == "ExternalInput":
            if nc.partition_id_tensor is not None and name == nc.partition_id_tensor.name:
                continue
            in_names.append(name)
        elif alloc.kind == "ExternalOutput":
            out_names.append(name)
            shape = tuple(alloc.tensor_shape)
            dtype = _mybir.dt.np(alloc.dtype)
            out_avals.append(jax.core.ShapedArray(shape, dtype))
            zero_outs.append(np.zeros((n_cores * shape[0], *shape[1:]), dtype))
    n_params = len(in_names)
    all_names = in_names + out_names
    if nc.partition_id_tensor is not None:
        all_names = all_names + [nc.partition_id_tensor.name]

    def _body(*args):
        operands = list(args)
        if nc.partition_id_tensor is not None:
            operands.append(bass2jax.partition_id_tensor())
        outs = bass2jax._bass_exec_p.bind(
            *operands,
            out_avals=tuple(out_avals),
            in_names=tuple(all_names),
            out_names=tuple(out_names),
            lowering_input_output_aliases=(),
            sim_require_finite=True,
            sim_require_nnan=True,
            nc=nc,
        )
        return tuple(outs)

    donate = tuple(range(n_params, n_params + len(out_names)))
    devices = jax.devices()[:n_cores]
    mesh = Mesh(np.asarray(devices), ("core",))
    sharded = jax.jit(
        shard_map(_body, mesh=mesh, in_specs=(PartitionSpec("core"),) * (n_params + len(out_names)),
                  out_specs=(PartitionSpec("core"),) * len(out_names), check_rep=False),
        donate_argnums=donate, keep_unused=True,
    )
    sh = NamedSharding(mesh, PartitionSpec("core"))
    dev_in = [
        jax.device_put(np.concatenate([np.asarray(in_maps[c][nm]) for c in range(n_cores)], axis=0), sh)
        for nm in in_names
    ]
    times = []
    for it in range(iters + 2):
        zo = [jax.device_put(z, sh) for z in zero_outs]
        for z in zo:
            z.block_until_ready()
        t0 = time.perf_counter()
        outs = sharded(*dev_in, *zo)
        for o in outs:
            o.block_until_ready()
        dt = time.perf_counter() - t0
        if it >= 2:
            times.append(dt)
    return times


def kernel(**inputs):
    (h, e), _ = _run(inputs, trace=False)
    return h, e
